# revision 4
# baseline (speedup 1.0000x reference)
"""Trainium2 Bass kernel for nn_MultiHeadAttention_7516192768586 (v2, fp8).

Full MHA: QKV projection -> masked softmax attention -> merge heads ->
residual add -> LayerNorm.  B=2, T=2048, D=1024, 16 heads (depth 64).

Sharding (8 cores): 2 batches x 4 head-groups (4 heads each = 256 channels).
All matmuls bf16 (fp8 fails the 2e-2 gate: scores and sharp-softmax V values
are too quantization-sensitive).  ctx is computed TRANSPOSED [t, 65] with e
stationary: full 128-partition output (2x fewer PE rows than the [65, t]
orientation) and no ctx transpose afterwards; column 64 per head is the
softmax denominator (ones column of V).
Masked keys are compacted away on host; pad keys have V=0 and ones-row=0 so
they contribute nothing to either numerator or denominator (no padbias).
LayerNorm stats (sum x, sum x^2) AllReduce within each batch-group of 4
cores, per 512-row chunk.  Residual q+bv fed bf16, output returned bf16.
"""

import sys

if "/opt/trn_rl_repo" not in sys.path:
    sys.path.insert(0, "/opt/trn_rl_repo")

import contextlib

import ml_dtypes
import numpy as np

import bass_rust as _br
import concourse.bass as bass
import concourse.tile as tile
from concourse import mybir
from concourse.bass_utils import run_bass_kernel_spmd
from concourse.vector_clock import ScopedClock

F32 = mybir.dt.float32
BF16 = mybir.dt.bfloat16
FP8 = mybir.dt.float8e4
BF = ml_dtypes.bfloat16
E4 = ml_dtypes.float8_e4m3

NUM_HEADS = 16
LN_EPS = 1e-5
B, T, D = 2, 2048, 1024
DEPTH = D // NUM_HEADS  # 64
HPC = 4  # heads per core
DD = HPC * DEPTH  # 256 projected channels per core
DV = 1152  # v contraction padded: 1024 + ones row -> 9 k-tiles of 128
AluOp = mybir.AluOpType
Act = mybir.ActivationFunctionType
DR = mybir.MatmulPerfMode.DoubleRow


class _TC(tile.TileContext):
    """TileContext whose tail drain splits its sem waits across 1-wait NOPs
    (this walrus build rejects >1 sync wait on one instruction)."""

    def _drain_and_barrier(self, tick_clock, wait_clock):
        nc = self.nc
        drain_inst = nc.sync.drain()
        wait_clock.add_sem_waits(
            drain_inst.ins, ScopedClock({None: tick_clock.global_clock})
        )
        si = drain_inst.ins.sync_info
        waits = list(si.on_wait) if si is not None and si.on_wait else []
        if len(waits) > 1:
            si.on_wait = waits[:1]
            for i in range(1, len(waits)):
                extra = nc.sync.nop()
                extra.ins.sync_info = _br.SyncInfo(
                    on_wait=waits[i : i + 1], on_update=[]
                )
        nc.all_engine_barrier()
        popped = nc._tile_sem_poison_stack.pop()
        assert popped is self._sem_poison
        assert self.sems is not None
        nc.clear_and_free_semaphores(list(self.sems.allocated().values()))
        nc.all_engine_barrier()


def _split_multi_waits(nc):
    """Move extra sem waits (>1 per instruction) onto same-engine NOPs
    inserted immediately before the instruction."""
    f = nc.m.functions[0]
    cur_bb = nc.cur_bb
    for block in f.blocks:
        insts = list(block.instructions)
        if not any(
            i.sync_info is not None
            and i.sync_info.on_wait
            and len(i.sync_info.on_wait) > 1
            for i in insts
        ):
            continue
        new_list = []
        for inst in insts:
            si = inst.sync_info
            if si is not None and si.on_wait and len(si.on_wait) > 1:
                waits = list(si.on_wait)
                si.on_wait = waits[:1]
                for w in waits[1:]:
                    eng = nc.engines[inst.engine]
                    nop = eng.nop()
                    tail_bb = cur_bb.bb if hasattr(cur_bb, "bb") else cur_bb
                    tl = list(tail_bb.instructions)
                    assert tl and tl[-1].name == nop.ins.name
                    tail_bb.instructions = tl[:-1]
                    nop.ins.sync_info = _br.SyncInfo(on_wait=[w], on_update=[])
                    new_list.append(nop.ins)
            new_list.append(inst)
        block.instructions = new_list


def _dup2(ap, n):
    """Insert a 0-stride dim of size 2 after the partition dim of a 2D AP."""
    return bass.AP(
        tensor=ap.tensor,
        offset=ap.offset,
        ap=[list(ap.ap[0]), [0, 2], [list(ap.ap[-1])[0], n]],
    )


def _bcast(ap, n):
    """Append a 0-stride dim of size n to an AP."""
    return bass.AP(
        tensor=ap.tensor,
        offset=ap.offset,
        ap=[list(d) for d in ap.ap] + [[0, n]],
    )


def _build(SP):
    """Build the per-core Bass program. SP = padded compacted key count."""
    NS = SP // 128  # s-tiles
    NP = NS // 2  # full exp pairs
    NPX = (NS + 1) // 2  # exp groups (last may be a single)
    NSCH = (SP + 511) // 512  # 512-col chunks of SP

    nc = bass.Bass("TRN2", target_bir_lowering=False, debug=False, num_devices=8)

    p = lambda name, shape, dt: nc.declare_dram_parameter(name, shape, dt, isOutput=False)
    # all big inputs pre-laid host-side in device tile order (partition-major)
    qTp = [p(f"qT{t}", [128, 8, 512], BF16) for t in range(4)]
    kTp = [p(f"kT{s}", [128, 8, min(512, SP - 512 * s)], BF16) for s in range(NSCH)]
    vTp = [p(f"vT{s}", [128, 8, min(512, SP - 512 * s)], BF16) for s in range(NSCH)]
    wqp = p("wqp", [128, 8, 256], BF16)
    wkp = p("wkp", [128, 8, 256], BF16)
    wvp = p("wvp", [128, 8, 260], BF16)
    msk = p("msk", [128, 16], BF16)  # [p, st] = 1 if key 128*st+p kept else 0
    bq2 = p("bq2", [128, 2], F32)
    bk2 = p("bk2", [128, 2], F32)
    qres = p("qres", [128, 16, 256], BF16)
    out = nc.declare_dram_parameter("out", [128, 16, 256], BF16, isOutput=True)

    with _TC(nc) as tc, contextlib.ExitStack() as ctx:
        singles = ctx.enter_context(tc.tile_pool(name="singles", bufs=1))
        persist = ctx.enter_context(tc.tile_pool(name="persist", bufs=1))
        work = ctx.enter_context(tc.tile_pool(name="work", bufs=8))
        epool = ctx.enter_context(tc.tile_pool(name="epool", bufs=10))
        dram = ctx.enter_context(tc.tile_pool(name="dram", bufs=1, space="DRAM"))
        psc = ctx.enter_context(tc.tile_pool(name="psc", bufs=2, space="PSUM"))
        pct = None  # ctx PSUM pool opened after v_proj's closes

        # ---- weights / small constants ----
        bk_sb = singles.tile([128, 2], F32)
        nc.scalar.dma_start(out=bk_sb[:], in_=bk2[:])
        bq_sb = singles.tile([128, 2], F32)
        nc.scalar.dma_start(out=bq_sb[:], in_=bq2[:])
        eps_sb = singles.tile([128, 1], F32)
        nc.vector.memset(eps_sb[:], LN_EPS)
        zero_sb = singles.tile([128, 1], F32)
        nc.vector.memset(zero_sb[:], 0.0)

        wk_sb = singles.tile([128, 8, 256], BF16)
        nc.sync.dma_start(out=wk_sb[:, 0:4, :], in_=wkp[:, 0:4, :])
        nc.sync.dma_start(out=wk_sb[:, 4:8, :], in_=wkp[:, 4:8, :])
        kin = []
        for s in range(NSCH):
            w = min(512, SP - 512 * s)
            t_ = singles.tile([128, 8, w], BF16, tag=f"kin{s}", name=f"kin{s}")
            nc.sync.dma_start(out=t_[:, 0:4, :], in_=kTp[s][:, 0:4, :])
            nc.sync.dma_start(out=t_[:, 4:8, :], in_=kTp[s][:, 4:8, :])
            kin.append(t_)
        wq_sb = singles.tile([128, 8, 256], BF16)
        nc.sync.dma_start(out=wq_sb[:], in_=wqp[:])
        qin = []
        for t in range(4):
            t_ = singles.tile([128, 8, 512], BF16, tag=f"qin{t}", name=f"qin{t}")
            nc.sync.dma_start(out=t_[:], in_=qTp[t][:])
            qin.append(t_)
        msk_sb = singles.tile([128, 16], BF16)
        nc.scalar.dma_start(out=msk_sb[:], in_=msk[:])
        wv_sb = singles.tile([128, 8, 260], BF16)
        nc.sync.dma_start(out=wv_sb[:], in_=wvp[:])
        vin = []
        for s in range(NSCH):
            w = min(512, SP - 512 * s)
            t_ = singles.tile([128, 8, w], BF16, tag=f"vin{s}", name=f"vin{s}")
            nc.sync.dma_start(out=t_[:], in_=vTp[s][:])
            vin.append(t_)
        qres_sb = persist.tile([128, 16, 256], BF16)
        nc.sync.dma_start(out=qres_sb[:], in_=qres[:])

        # ---- persistent activations ----
        QT_sb = persist.tile([128, 2, T], BF16)  # [64*(h%2)+d, h//2, t]
        KT_sb = persist.tile([128, 2, SP], BF16)
        VH_sb = persist.tile([128, NS, 260], BF16)  # [s, st, 65h+c]
        x_sb = persist.tile([128, 16, 256], BF16)
        mu_sb = singles.tile([128, 16], F32)
        rstd_sb = singles.tile([128, 16], F32)
        stats_dram = [dram.tile([128, 8], F32, name=f"std{i}") for i in range(4)]
        ar_dram = [dram.tile([128, 8], F32, name=f"ard{i}") for i in range(4)]
        stats_sb = [singles.tile([128, 8], F32, name=f"sth{i}") for i in range(4)]

        def k_proj(ddt):
            if True:
                for sch in range(NSCH):
                    w = min(512, SP - 512 * sch)
                    ps2 = psc.tile([128, 2, 512], F32, tag="sps", name="kps")
                    ps = ps2[:, 0, :]
                    for kt in range(8):
                        nc.tensor.matmul(
                            ps[:, :w],
                            wk_sb[:, kt, 128 * ddt : 128 * (ddt + 1)],
                            kin[sch][:, kt, :],
                            start=(kt == 0),
                            stop=(kt == 7),
                        )
                    nc.vector.tensor_scalar(
                        out=KT_sb[:, ddt, 512 * sch : 512 * sch + w],
                        in0=ps[:, :w],
                        scalar1=bk_sb[:, ddt : ddt + 1],
                        scalar2=None,
                        op0=AluOp.add,
                    )

        def q_proj(tch, ddts=(0, 1)):
            for ddt in ddts:
                ps2 = psc.tile([128, 2, 512], F32, tag="sps", name="qps")
                ps = ps2[:, 0, :]
                for kt in range(8):
                    nc.tensor.matmul(
                        ps[:],
                        wq_sb[:, kt, 128 * ddt : 128 * (ddt + 1)],
                        qin[tch][:, kt, :],
                        start=(kt == 0),
                        stop=(kt == 7),
                    )
                nc.vector.tensor_scalar(
                    out=QT_sb[:, ddt, 512 * tch : 512 * (tch + 1)],
                    in0=ps[:],
                    scalar1=bq_sb[:, ddt : ddt + 1],
                    scalar2=None,
                    op0=AluOp.add,
                )

        def v_proj(st_list):
            for st in st_list:
                sch, st4 = st // 4, st % 4
                ps2 = psc.tile([128, 2, 512], F32, tag="sps", name="vps")
                ps = ps2[:, 0, 0:260]
                for kt in range(8):
                    nc.tensor.matmul(
                        ps,
                        vin[sch][:, kt, 128 * st4 : 128 * (st4 + 1)],
                        wv_sb[:, kt, :],
                        start=(kt == 0),
                        stop=(kt == 7),
                    )
                nc.vector.tensor_copy(VH_sb[:, st, :], ps)
                ones_dst = bass.AP(
                    tensor=VH_sb.tensor,
                    offset=VH_sb[:, st, 64].offset,
                    ap=[list(VH_sb.ap[0]), [65, 4]],
                )
                nc.vector.tensor_copy(out=ones_dst, in_=_bcast(msk_sb[:, st : st + 1], 4)[:, 0, :])

        def attn_scores(tch, h):
            a, g = h % 2, h // 2
            t0 = 512 * tch
            es = []
            for stp in range(NPX):
                n_i = 2 if stp < NP else 1
                sps = psc.tile([128, 2, 512], F32, tag="sps")
                for i in range(n_i):
                    st = 2 * stp + i
                    nc.tensor.matmul(
                        sps[:, i, :],
                        KT_sb[64 * a : 64 * a + 64, g, 128 * st : 128 * (st + 1)],
                        QT_sb[64 * a : 64 * a + 64, g, t0 : t0 + 512],
                        start=True,
                        stop=True,
                    )
                e = epool.tile([128, 2, 512], BF16, tag="e")
                nc.scalar.activation(
                    out=e[:, :n_i, :],
                    in_=sps[:, :n_i, :],
                    func=Act.Exp,
                    bias=zero_sb[:, 0:1],
                    scale=0.125,
                )
                es.append(e)
            return es

        def attn_ctx(tch, h, es):
            cps = pct.tile([128, 4, 512], F32, tag="cps")
            for stp in range(NPX):
                n_i = 2 if stp < NP else 1
                e = es[stp]
                for i in range(n_i):
                    st = 2 * stp + i
                    for tt4 in range(4):
                        nc.tensor.matmul(
                            cps[:, tt4, 0:65],
                            e[:, i, 128 * tt4 : 128 * (tt4 + 1)],
                            VH_sb[:, st, 65 * h : 65 * (h + 1)],
                            start=(st == 0),
                            stop=(st == NS - 1),
                        )
            # LN partial: x[:, tt, 64h:64h+64] = ctx * (1/denom) + qres
            rinv = work.tile([128, 4], F32, tag="rinv")
            nc.vector.reciprocal(rinv[:], cps[:, :, 64:65])
            xx = work.tile([128, 4, 64], F32, tag="xx")
            nc.vector.tensor_tensor(
                out=xx[:], in0=cps[:, :, 0:64], in1=_bcast(rinv[:], 64), op=AluOp.mult
            )
            nc.vector.tensor_tensor(
                out=x_sb[:, 4 * tch : 4 * tch + 4, 64 * h : 64 * h + 64],
                in0=xx[:],
                in1=qres_sb[:, 4 * tch : 4 * tch + 4, 64 * h : 64 * h + 64],
                op=AluOp.add,
            )

        def attn_head(tch, h):
            attn_ctx(tch, h, attn_scores(tch, h))

        def stats(tch):
            st_t = stats_sb[tch]
            for tt4 in range(4):
                tt = 4 * tch + tt4
                nc.vector.tensor_reduce(
                    out=st_t[:, tt4 : tt4 + 1],
                    in_=x_sb[:, tt, :],
                    axis=mybir.AxisListType.X,
                    op=AluOp.add,
                )
                sq = work.tile([128, 256], BF16, tag="sq")
                nc.scalar.activation(
                    out=sq[:],
                    in_=x_sb[:, tt, :],
                    func=Act.Square,
                    accum_out=st_t[:, 4 + tt4 : 5 + tt4],
                )

        def stats_ar(tch):
            nc.sync.dma_start(out=stats_dram[tch][:], in_=stats_sb[tch][:])
            nc.gpsimd.collective_compute(
                "AllReduce",
                AluOp.add,
                replica_groups=[[0, 1, 2, 3], [4, 5, 6, 7]],
                ins=[stats_dram[tch][:].opt()],
                outs=[ar_dram[tch][:].opt()],
            )

        def phase7(tch):
            gst = work.tile([128, 8], F32, tag="gst", name=f"gst{tch}")
            nc.sync.dma_start(out=gst[:], in_=ar_dram[tch][:])
            c0 = 4 * tch
            nc.vector.tensor_scalar(
                out=mu_sb[:, c0 : c0 + 4], in0=gst[:, 0:4],
                scalar1=1.0 / D, scalar2=None, op0=AluOp.mult,
            )
            ex2 = work.tile([128, 4], F32, tag="ex2")
            nc.vector.tensor_scalar(
                out=ex2[:], in0=gst[:, 4:8], scalar1=1.0 / D, scalar2=None, op0=AluOp.mult
            )
            var = work.tile([128, 4], F32, tag="ex2")
            nc.vector.tensor_tensor(
                out=var[:], in0=mu_sb[:, c0 : c0 + 4], in1=mu_sb[:, c0 : c0 + 4],
                op=AluOp.mult,
            )
            nc.vector.tensor_tensor(out=var[:], in0=ex2[:], in1=var[:], op=AluOp.subtract)
            sd = work.tile([128, 4], F32, tag="ex2")
            nc.scalar.activation(out=sd[:], in_=var[:], func=Act.Sqrt, bias=eps_sb[:, 0:1], scale=1.0)
            nc.vector.reciprocal(rstd_sb[:, c0 : c0 + 4], sd[:])
            for tt4 in range(4):
                tt = c0 + tt4
                xn = work.tile([128, 256], BF16, tag="xn")
                nc.vector.tensor_scalar(
                    out=xn[:],
                    in0=x_sb[:, tt, :],
                    scalar1=mu_sb[:, tt : tt + 1],
                    scalar2=rstd_sb[:, tt : tt + 1],
                    op0=AluOp.subtract,
                    op1=AluOp.mult,
                )
                nc.sync.dma_start(out=out[:, tt, :], in_=xn[:])

        # warm up the collective framework (first cc op pays a big init
        # barrier) while the projections run
        warm = dram.tile([128, 1], F32, name="warm")
        warm_o = dram.tile([128, 1], F32, name="warmo")
        wsb = singles.tile([128, 1], F32)
        nc.vector.memset(wsb[:], 0.0)
        nc.sync.dma_start(out=warm[:], in_=wsb[:])
        nc.gpsimd.collective_compute(
            "AllReduce",
            AluOp.add,
            replica_groups=[[0, 1, 2, 3], [4, 5, 6, 7]],
            ins=[warm[:].opt()],
            outs=[warm_o[:].opt()],
        )
        pct = ctx.enter_context(tc.tile_pool(name="pct", bufs=1, space="PSUM"))
        k_proj(0)
        q_proj(0, (0,))
        es00 = attn_scores(0, 0)  # h0 exp starts asap
        q_proj(0, (1,))
        k_proj(1)
        v_proj(range(NS))
        attn_ctx(0, 0, es00)
        for h in range(1, HPC):
            attn_head(0, h)
        stats(0)
        stats_ar(0)
        q_proj(1)
        for tch in range(1, 4):
            for h in range(HPC):
                attn_head(tch, h)
            stats(tch)
            stats_ar(tch)
            if tch < 3:
                q_proj(tch + 1)
                phase7(tch - 1)
        phase7(2)
        phase7(3)
    _split_multi_waits(nc)
    return nc


_CACHE = {}
_LAST_IN_MAPS = None
_LAST_RES = None


def kernel(q, k, v, mask, causality, edge_fea, wq, bq, wk, bk, wv, bv, gamma, beta):
    # NB: the reference masks attention row (head eta, batch beta) with
    # mask[eta // 8]; with 4 heads per core this is mask[hg // 2].
    q = np.asarray(q, np.float32)
    k = np.asarray(k, np.float32)
    v = np.asarray(v, np.float32)
    mask = np.asarray(mask)
    wq = np.asarray(wq, np.float32)
    bq = np.asarray(bq, np.float32)
    wk = np.asarray(wk, np.float32)
    bk = np.asarray(bk, np.float32)
    wv = np.asarray(wv, np.float32)
    bv = np.asarray(bv, np.float32)
    gamma = np.asarray(gamma, np.float32)
    beta = np.asarray(beta, np.float32)
    assert int(np.asarray(causality)) == 0

    keep = [np.flatnonzero(mask[g] == 0) for g in range(2)]
    slens = [len(kp) for kp in keep]
    SP = max(256, ((max(slens) + 127) // 128) * 128)
    NSCH = (SP + 511) // 512

    def pack_kt(mat, nkt):
        # [R, C] (R = nkt*128) -> [128, nkt, C] device layout
        R, C = mat.shape
        assert R == 128 * nkt
        return np.ascontiguousarray(mat.reshape(nkt, 128, C).transpose(1, 0, 2))

    # q^T per batch: [1024, 2048] bf16, k-tile layout, split in 4 t-chunks
    qT8 = {}
    for b in range(2):
        full = pack_kt(q[b].T.astype(BF), 8)  # [128, 8, 2048]
        qT8[b] = [np.ascontiguousarray(full[:, :, 512 * t : 512 * (t + 1)]) for t in range(4)]
    kT8, vT8 = {}, {}
    for b in range(2):
        for g in range(2):
            kk = np.zeros((1024, SP), BF)
            kk[:, : slens[g]] = k[b][keep[g]].T.astype(BF)
            kp_full = pack_kt(kk, 8)
            kT8[b, g] = [
                np.ascontiguousarray(kp_full[:, :, 512 * s : min(512 * (s + 1), SP)])
                for s in range(NSCH)
            ]
            vv = np.zeros((D, SP), BF)
            vv[:, : slens[g]] = v[b][keep[g]].T.astype(BF)
            vp_full = pack_kt(vv, 8)
            vT8[b, g] = [
                np.ascontiguousarray(vp_full[:, :, 512 * s : min(512 * (s + 1), SP)])
                for s in range(NSCH)
            ]

    in_maps = []
    for c in range(8):
        b, hg = c // 4, c % 4
        g = hg // 2
        c0 = hg * DD
        wvp = np.zeros((D, HPC * 65), BF)
        for hh in range(HPC):
            wvp[:D, 65 * hh : 65 * hh + 64] = (
                wv[c0 + 64 * hh : c0 + 64 * (hh + 1)].T.astype(BF)
            )
        mk = np.zeros((SP,), np.float32)
        mk[: slens[g]] = 1.0
        mka = np.zeros((128, 16), BF)
        mka[:, : SP // 128] = mk.reshape(-1, 128).T.astype(BF)
        qres_h = (q[b][:, c0 : c0 + DD] + bv[c0 : c0 + DD]).astype(BF)
        im = {
            "wqp": pack_kt(np.ascontiguousarray(wq[c0 : c0 + DD].T).astype(BF), 8),
            "wkp": pack_kt(np.ascontiguousarray(wk[c0 : c0 + DD].T).astype(BF), 8),
            "wvp": pack_kt(wvp, 8),
            "msk": mka,
            "bq2": np.ascontiguousarray(bq[c0 : c0 + DD].reshape(2, 128).T),
            "bk2": np.ascontiguousarray(bk[c0 : c0 + DD].reshape(2, 128).T),
            "qres": np.ascontiguousarray(
                qres_h.reshape(16, 128, 256).transpose(1, 0, 2)
            ),
        }
        for t in range(4):
            im[f"qT{t}"] = qT8[b][t]
        for s in range(NSCH):
            im[f"kT{s}"] = kT8[b, g][s]
            im[f"vT{s}"] = vT8[b, g][s]
        in_maps.append(im)

    global _LAST_IN_MAPS
    _LAST_IN_MAPS = in_maps
    if SP not in _CACHE:
        _CACHE[SP] = _build(SP)
    nc = _CACHE[SP]

    res = run_bass_kernel_spmd(nc, in_maps, list(range(8))).results
    global _LAST_RES
    _LAST_RES = res

    full = np.empty((B, T, D), np.float32)
    for c in range(8):
        b, hg = c // 4, c % 4
        o = res[c]["out"]  # [128, 16, 256] bf16 (normalized, pre-gamma/beta)
        full[b, :, hg * DD : (hg + 1) * DD] = (
            np.asarray(o).astype(np.float32).transpose(1, 0, 2).reshape(T, DD)
            * gamma[hg * DD : (hg + 1) * DD]
            + beta[hg * DD : (hg + 1) * DD]
        )
    return full
